# revision 18
# baseline (speedup 1.0000x reference)
"""Bass/Tile kernel for a 3-layer bidirectional LSTM classifier on 8 TRN2 cores.

Problem shapes (hardcoded): x [256, 512, 16], H=256, 3 BiLSTM layers, fc -> [256].

Strategy: data-parallel over batch (B=32 per core, no collectives). Per core,
the fwd and rev recurrences of each layer run as two interleaved streams. All
state is kept transposed (hT [H, B], gates [4H, B]) so every matmul keeps
weights as the PE-stationary operand. Gate rows are pre-permuted host-side to
[i, f, o, g] chunk order so one sigmoid covers chunks 0..5 and one tanh covers
chunks 6..7. Everything on the PE path is bf16 (FWL weight loads, 1 cycle/row);
cell state c stays fp32 and the final h that feeds the classifier is fp32.

v5: the input projection never goes to DRAM and never forms a separate phase.
For each 16-step block it is computed one block ahead at N=512 (weight loads
amortized over 16 steps) into a PSUM bank as one ATOMIC accumulation group per
gate chunk — PSUM groups must be strictly sequential: interleaving two open
groups corrupts earlier group state (measured), which is why the projection
cannot share the per-step group with the recurrent matmuls. A gpsimd
tensor_scalar pass folds the per-chunk bias while copying PSUM -> SBUF bf16.
The per-step PSUM group holds only the two recurrent matmuls per gate chunk;
the DVE adds projection + recurrent gates, and the activations read SBUF.
The projection m-groups and copies are spread one per step so they execute
inside the previous steps' chain latency; xp block buffers ping-pong across
blocks (loop unrolled by 2)."""

import os
from contextlib import ExitStack

import numpy as np
import ml_dtypes

import concourse.bass as bass
import concourse.mybir as mybir
import concourse.tile as tile
from concourse import bacc, bass_utils
from concourse.bass import ds

f32 = mybir.dt.float32
bf16 = mybir.dt.bfloat16
AF = mybir.ActivationFunctionType
np_bf16 = ml_dtypes.bfloat16

H = 256
NCORES = 8
BFULL = 256
TFULL = 512
I0 = 16

# gate chunk order i,i,f,f,o,o,g,g (PyTorch order in weights is i,f,g,o)
_PERM = np.concatenate(
    [np.arange(0, 512), np.arange(768, 1024), np.arange(512, 768)]
)
# per-m emission order: g-chunks first so the tanh can start early
_MORD = (6, 7, 0, 1, 2, 3, 4, 5)


def _prep_wih(w):
    """[1024, Din] -> stationary layout [min(Din,128), nk*1024], chunk (k, m)
    at cols k*1024 + m*128; lhsT[kk, m*128+mm] = w_perm[m*128+mm, k*128+kk]."""
    wr = np.asarray(w, np.float32)[_PERM]
    din = wr.shape[1]
    if din <= 128:
        return np.ascontiguousarray(wr.T.astype(np_bf16))
    nk = din // 128
    out = np.empty((128, nk * 1024), np_bf16)
    for k in range(nk):
        out[:, k * 1024 : (k + 1) * 1024] = wr[:, k * 128 : (k + 1) * 128].T
    return out


def _prep_b(b):
    return np.ascontiguousarray(np.asarray(b, np.float32)[_PERM].reshape(8, 128).T)


def build(nc, T=TFULL, B=32):
    """Emit the full per-core program into nc (a Bacc)."""
    TB = T * B
    U = 16
    NBLK = T // U
    UB = U * B

    xT0 = nc.dram_tensor("xT0", [I0, TB], bf16, kind="ExternalInput").ap()
    win = {}
    for l in range(3):
        kp = I0 if l == 0 else 128
        kch = 1 if l == 0 else 4
        for d, dn in enumerate("fr"):
            win[(l, d, "wih")] = nc.dram_tensor(
                f"wih{l}{dn}", [kp, kch * 1024], bf16, kind="ExternalInput"
            ).ap()
            win[(l, d, "whh")] = nc.dram_tensor(
                f"whh{l}{dn}", [128, 2048], bf16, kind="ExternalInput"
            ).ap()
            win[(l, d, "b")] = nc.dram_tensor(
                f"b{l}{dn}", [128, 8], f32, kind="ExternalInput"
            ).ap()
    b2rb = nc.dram_tensor("b2rb", [128, 8 * B], f32, kind="ExternalInput").ap()
    identin = nc.dram_tensor("ident", [128, 128], bf16, kind="ExternalInput").ap()
    out_h2f = nc.dram_tensor("h2f", [128, 2 * B], f32, kind="ExternalOutput").ap()
    out_h2r = nc.dram_tensor("h2r", [128, 2 * B], f32, kind="ExternalOutput").ap()

    with tile.TileContext(nc) as tc, ExitStack() as ctx:
        dram = ctx.enter_context(tc.tile_pool(name="dram", bufs=1, space="DRAM"))
        wpool = ctx.enter_context(tc.tile_pool(name="wts", bufs=1))
        slabpool = ctx.enter_context(tc.tile_pool(name="slab", bufs=1))
        pj = [
            ctx.enter_context(tc.tile_pool(name=f"pj{d}", bufs=2, space="PSUM"))
            for d in range(2)
        ]
        rpsB = [
            ctx.enter_context(tc.tile_pool(name=f"rps{d}", bufs=2, space="PSUM"))
            for d in range(2)
        ]
        gpool = ctx.enter_context(tc.tile_pool(name="g", bufs=3))
        state = ctx.enter_context(tc.tile_pool(name="st", bufs=1))
        tmp = ctx.enter_context(tc.tile_pool(name="tmp", bufs=3))

        xin = {
            1: dram.tile([4, 128, TB], bf16, tag="xin1", name="xin1"),
            2: dram.tile([4, 128, TB], bf16, tag="xin2", name="xin2"),
        }
        ident = wpool.tile([128, 128], bf16, tag="ident")
        nc.sync.dma_start(ident[:], identin[:])

        def load_weights(l):
            kp = I0 if l == 0 else 128
            kch = 1 if l == 0 else 4
            wt = {}
            for d in range(2):
                wih_t = wpool.tile([kp, kch * 1024], bf16, tag=f"wih{d}")
                nc.sync.dma_start(wih_t[:], win[(l, d, "wih")][:])
                whh_t = wpool.tile([128, 2048], bf16, tag=f"whh{d}")
                nc.sync.dma_start(whh_t[:], win[(l, d, "whh")][:])
                b_t = wpool.tile([128, 8], f32, tag=f"b{d}")
                nc.sync.dma_start(b_t[:], win[(l, d, "b")][:])
                wt[d] = (wih_t, whh_t, b_t)
            return wt

        def load_slab_set(l, jb, par):
            """Stage the input columns block jb's projection needs (both
            directions). p=0 holds input block jb, p=1 block NBLK-1-jb."""
            sl = {}
            kk = 1 if l == 0 else 4
            kp = I0 if l == 0 else 128
            for k in range(kk):
                for p in range(2):
                    t_ = slabpool.tile([kp, UB], bf16, tag=f"s{k}{p}{par}")
                    col = ds(jb * UB, UB) if p == 0 else ds(
                        (NBLK - 1) * UB - jb * UB, UB
                    )
                    src = xT0[:, col] if l == 0 else xin[l][k, :, col]
                    nc.sync.dma_start(t_[:], src)
                    sl[(k, p)] = t_
            return sl

        def proj_mgroup(l, d, wt, sl, xpb, m):
            """One ATOMIC projection accumulation group (gate chunk m, all 16
            steps of a block at N=512), then a bias-fused copy to SBUF bf16."""
            wih_t, _, b_t = wt[d]
            ps = pj[d].tile([128, UB], f32, tag="pjps")
            if l == 0:
                p = 0 if d == 0 else 1
                slab3 = sl[(0, p)][:].rearrange("p (u b) -> p u b", b=B)
                rhs = slab3 if d == 0 else slab3[:, ::-1, :]
                nc.tensor.matmul(
                    ps[:],
                    wih_t[:, m * 128 : (m + 1) * 128],
                    rhs,
                    start=True,
                    stop=True,
                )
            else:
                for k in range(4):
                    straight = (d == 0) if k < 2 else (d == 1)
                    p = 0 if straight else 1
                    slab3 = sl[(k, p)][:].rearrange("p (u b) -> p u b", b=B)
                    rhs = slab3 if straight else slab3[:, ::-1, :]
                    nc.tensor.matmul(
                        ps[:],
                        wih_t[:, (k * 8 + m) * 128 : (k * 8 + m + 1) * 128],
                        rhs,
                        start=(k == 0),
                        stop=(k == 3),
                    )
            nc.vector.tensor_scalar_add(
                xpb[:, m, :, :],
                ps[:].rearrange("p (u b) -> p u b", b=B),
                b_t[:, m : m + 1],
            )

        def step_h_group(d, wt, s, hhist, psB):
            _, whh_t, _ = wt[d]
            sp = U - 1 if s == 0 else s - 1
            for m in _MORD:
                for k in range(2):
                    nc.tensor.matmul(
                        psB[:, m, :],
                        whh_t[:, (k * 8 + m) * 128 : (k * 8 + m + 1) * 128],
                        hhist[:, k, sp, :],
                        start=False,
                        stop=(k == 1),
                    )

        def cell_chain(d, s, psB, xpb, hhist, cc, fin=None):
            gf = psB[:].rearrange("p m b -> p (m b)")
            tg = gpool.tile([128, 2 * B], f32, tag=f"tg{d}")
            nc.scalar.activation(tg[:], gf[:, 6 * B : 8 * B], AF.Tanh)
            sg = gpool.tile([128, 6 * B], f32, tag=f"sg{d}")
            nc.scalar.activation(sg[:], gf[:, 0 : 6 * B], AF.Sigmoid)
            ta = tmp.tile([128, 2 * B], f32, tag=f"ta{d}")
            nc.gpsimd.tensor_mul(ta[:], sg[:, 2 * B : 4 * B], cc[:])  # f*c
            tb = tmp.tile([128, 2 * B], f32, tag=f"tb{d}")
            nc.vector.tensor_mul(tb[:], sg[:, 0 : 2 * B], tg[:])  # i*g
            nc.vector.tensor_add(cc[:], ta[:], tb[:])
            tcb = tmp.tile([128, 2 * B], f32, tag=f"tc{d}")
            nc.scalar.activation(tcb[:], cc[:], AF.Tanh)
            hv = hhist[:, :, s, :]
            og = sg[:, 4 * B : 6 * B].rearrange("p (k b) -> p k b", b=B)
            tc2 = tcb[:].rearrange("p (k b) -> p k b", b=B)
            nc.gpsimd.tensor_mul(hv, og, tc2)  # o*tanh(c) -> bf16 h
            if fin is not None:
                nc.vector.tensor_mul(fin[:], sg[:, 4 * B : 6 * B], tcb[:])

        def rec_block(l, dirs, wt, hh, cs, xpb, jb, par, prefetch, store,
                      fin=None):
            if prefetch:
                sln = load_slab_set(l, jb + 1, 1 - par)
            for s in range(U):
                pss = {}
                for d in dirs:
                    psB = rpsB[d].tile([128, 8, B], f32, tag="psB")
                    nc.tensor.matmul(
                        psB[:],
                        ident[:],
                        xpb[d][par][:, :, s, :],
                        start=True,
                        stop=False,
                    )
                    step_h_group(d, wt, s, hh[d], psB)
                    pss[d] = psB
                for d in dirs:
                    f = (
                        fin
                        if (fin is not None and s == U - 1 and d == 0)
                        else None
                    )
                    cell_chain(d, s, pss[d], xpb[d][par], hh[d], cs[d], fin=f)
                if prefetch:
                    if len(dirs) == 2:
                        d_, m_ = s % 2, s // 2
                    else:
                        d_, m_ = dirs[0], (s // 2 if s % 2 == 0 else None)
                    if m_ is not None:
                        proj_mgroup(l, d_, wt, sln, xpb[d_][1 - par], m_)
            if store:
                for d in dirs:
                    for k in range(2):
                        nc.sync.dma_start(
                            xin[l + 1][2 * d + k, :, ds(jb * UB, UB)],
                            hh[d][:, k, :, :].rearrange("p u b -> p (u b)"),
                        )

        def rec_layer(l, wt, dirs=(0, 1), store=True, fin=None):
            hh, cs, xpb = {}, {}, {}
            for d in dirs:
                hhist = state.tile([128, 2, U, B], bf16, tag=f"h{d}")
                cc = state.tile([128, 2 * B], f32, tag=f"c{d}")
                nc.gpsimd.memset(hhist[:], 0.0)
                nc.gpsimd.memset(cc[:], 0.0)
                hh[d], cs[d] = hhist, cc
                xpb[d] = {
                    0: state.tile(
                        [128, 8, U, B], bf16, tag=f"xpA{d}", name=f"xpA{d}"
                    ),
                    1: state.tile(
                        [128, 8, U, B], bf16, tag=f"xpB{d}", name=f"xpB{d}"
                    ),
                }
            # prologue: block 0's projection
            sl0 = load_slab_set(l, 0, 0)
            for d in dirs:
                for m in _MORD:
                    proj_mgroup(l, d, wt, sl0, xpb[d][0], m)
            args = (l, dirs, wt, hh, cs, xpb)
            with tc.For_i(
                0, (NBLK - 2) // 2, 1, hint_engines=(mybir.EngineType.PE,)
            ) as jb2:
                rec_block(*args, 2 * jb2, 0, True, store)
                rec_block(*args, 2 * jb2 + 1, 1, True, store)
            rec_block(*args, NBLK - 2, 0, True, store)
            rec_block(*args, NBLK - 1, 1, False, store, fin=fin)
            return hh, cs

        # ---- layers ----
        wt = load_weights(0)
        rec_layer(0, wt)
        wt = load_weights(1)
        rec_layer(1, wt)
        wt = load_weights(2)
        hfin = state.tile([128, 2 * B], f32, tag="hfin")
        rec_layer(2, wt, dirs=(0,), store=False, fin=hfin)
        nc.sync.dma_start(out_h2f[:], hfin[:])

        # layer-2 reverse: only its first step (t = T-1) feeds the output.
        # h_prev = c_prev = 0 so gates = Wih_r . x2(T-1) + b and c = i*g.
        wih_t, _, _ = wt[1]
        b2 = wpool.tile([128, 8 * B], f32, tag="b2rb")
        nc.sync.dma_start(b2[:], b2rb[:])
        xs = {}
        for k in range(4):
            t_ = slabpool.tile([128, B], bf16, tag=f"l2r{k}")
            col = ds((NBLK - 1) * UB + (U - 1) * B, B) if k < 2 else ds(0, B)
            nc.sync.dma_start(t_[:], xin[2][k, :, col])
            xs[k] = t_
        psr = rpsB[1].tile([128, 8, B], f32, tag="psB")
        for m in range(8):
            for k in range(4):
                nc.tensor.matmul(
                    psr[:, m, :],
                    wih_t[:, (k * 8 + m) * 128 : (k * 8 + m + 1) * 128],
                    xs[k][:],
                    start=(k == 0),
                    stop=(k == 3),
                )
        g = gpool.tile([128, 8 * B], f32, tag="l2r_g")
        nc.vector.tensor_add(g[:], psr[:].rearrange("p m b -> p (m b)"), b2[:])
        sg = gpool.tile([128, 6 * B], f32, tag="l2r_sg")
        nc.scalar.activation(sg[:], g[:, 0 : 6 * B], AF.Sigmoid)
        tg = gpool.tile([128, 2 * B], f32, tag="l2r_tg")
        nc.scalar.activation(tg[:], g[:, 6 * B : 8 * B], AF.Tanh)
        cr = state.tile([128, 2 * B], f32, tag="l2r_c")
        nc.vector.tensor_mul(cr[:], sg[:, 0 : 2 * B], tg[:])  # c = i*g
        tcb = tmp.tile([128, 2 * B], f32, tag="l2r_tc")
        nc.scalar.activation(tcb[:], cr[:], AF.Tanh)
        hr = state.tile([128, 2 * B], f32, tag="l2r_h")
        nc.vector.tensor_mul(hr[:], sg[:, 4 * B : 6 * B], tcb[:])
        nc.sync.dma_start(out_h2r[:], hr[:])


def _make_in_maps(inputs, T=TFULL, B=32, ncores=NCORES):
    x = np.asarray(inputs["x"], np.float32)
    shared = {}
    for l in range(3):
        for d, dn in enumerate("fr"):
            shared[f"wih{l}{dn}"] = _prep_wih(inputs[f"wih{l}{dn}"])
            shared[f"whh{l}{dn}"] = _prep_wih(inputs[f"whh{l}{dn}"])
            shared[f"b{l}{dn}"] = _prep_b(inputs[f"b{l}{dn}"])
    shared["b2rb"] = np.ascontiguousarray(
        np.repeat(shared["b2r"], B, axis=1).astype(np.float32)
    )
    shared["ident"] = np.ascontiguousarray(np.eye(128, dtype=np_bf16))
    in_maps = []
    for ci in range(ncores):
        xs = x[ci * B : (ci + 1) * B, :T]  # [B, T, 16]
        xt = xs.transpose(2, 1, 0).reshape(I0, T * B)
        m = dict(shared)
        m["xT0"] = np.ascontiguousarray(xt.astype(np_bf16))
        in_maps.append(m)
    return in_maps


def _assemble(results, inputs, B=32):
    fcw = np.asarray(inputs["fcw"], np.float32)[0]
    fcb = float(np.asarray(inputs["fcb"], np.float32)[0])
    out = np.empty(len(results) * B, np.float32)
    for ci, r in enumerate(results):
        h2f = np.concatenate([r["h2f"][:, :B], r["h2f"][:, B:]], axis=0)
        h2r = np.concatenate([r["h2r"][:, :B], r["h2r"][:, B:]], axis=0)
        out[ci * B : (ci + 1) * B] = fcw[:256] @ h2f + fcw[256:] @ h2r + fcb
    return out


def kernel(**inputs):
    nc = bacc.Bacc(
        "TRN2", target_bir_lowering=False, debug=False, num_devices=NCORES
    )
    build(nc)
    nc.compile()
    in_maps = _make_in_maps(inputs)
    trace = os.environ.get("KERNEL_TRACE", "0") == "1"
    res = bass_utils.run_bass_kernel_spmd(
        nc,
        in_maps,
        core_ids=list(range(NCORES)),
        trace=trace,
        tmpdir=os.environ.get("KERNEL_TRACE_DIR") if trace else None,
    )
    if trace and res.exec_time_ns is not None:
        print(f"HW exec time: {res.exec_time_ns} ns")
    return _assemble(res.results, inputs)
